# revision 2
# baseline (speedup 1.0000x reference)
"""Trainium2 Bass kernel for nn_MetapathEmbed.

Math:  out[b,m] = sum_{c,d,e} pools[b,d] * metapath[c,d] * card[c,e] * kern[e,m]
Factored:
    X = card @ kern                  [C, M]
    W = metapath^T-contraction:  W[d,m] = sum_c metapath[c,d] * X[c,m]
    out = pools @ W                  [B, M]

Sharding: metapath (and card) row-sharded over c across 8 cores; each core
computes W_partial (its c-slice) and out_partial = pools @ W_partial [B, M];
host sums the 8 partials (associativity of the c-sum).

Per-core dataflow (all fp32):
  - X[c,m] tiles computed on-chip from host-pre-transposed cardT.
  - Main loop: W^T[m,d] accumulated in PSUM over c-chunks;
    lhsT = X tile [c,128m] (stationary), rhs = metapath tile [c,1024d]
    (moving, N=512 per matmul, the only full-rate fp32 arrangement).
  - W^T -> W via TensorE 128x128 transposes, then
    out_psum[b,m] += poolsT_tile^T @ W_tile over all 128 d-chunks.
poolsT and cardT are pre-transposed/tiled on the host (pure layout prep).
"""

import sys

if "/opt/trn_rl_repo" not in sys.path:
    sys.path.insert(0, "/opt/trn_rl_repo")

import numpy as np

B, C, E, M = 128, 16384, 64, 128
N_CORES = 8
R = C // N_CORES          # 2048 metapath rows per core
RC = R // 128             # 16 c-chunks per core
DG = 1024                 # d-group width (2 psum banks)
NG = C // DG              # 16 d-groups
TPG = DG // 128           # 8 transpose tiles per group
ND = C // 128             # 128 d-chunks total (final contraction)

_NC_CACHE = {}


def _build_nc():
    if "nc" in _NC_CACHE:
        return _NC_CACHE["nc"]

    from concourse import bacc, mybir
    from concourse.tile import TileContext
    from concourse.masks import make_identity

    dt = mybir.dt
    nc = bacc.Bacc(None, target_bir_lowering=False)

    mp_d = nc.declare_dram_parameter("mp", [R, C], dt.float32, isOutput=False)
    poolsT_d = nc.declare_dram_parameter("poolsT", [128, C], dt.float32, isOutput=False)
    cardT_d = nc.declare_dram_parameter("cardT", [E, R], dt.float32, isOutput=False)
    kern_d = nc.declare_dram_parameter("kern", [E, M], dt.float32, isOutput=False)
    out_d = nc.declare_dram_parameter("out", [B, M], dt.float32, isOutput=True)

    with TileContext(nc) as tc:
        with (
            tc.tile_pool(name="const", bufs=1) as const_pool,
            tc.tile_pool(name="big", bufs=1) as big_pool,
            tc.tile_pool(name="mp", bufs=6) as mp_pool,
            tc.tile_pool(name="stage", bufs=2) as stage_pool,
            tc.tile_pool(name="psA", bufs=2, space="PSUM") as psum_a,
            tc.tile_pool(name="psB", bufs=2, space="PSUM") as psum_b,
            tc.tile_pool(name="psT", bufs=2, space="PSUM") as psum_t,
            tc.tile_pool(name="psO", bufs=1, space="PSUM") as psum_o,
        ):
            # ---- Phase A: constants + X = card @ kern ----
            ident = const_pool.tile([128, 128], dt.float32)
            make_identity(nc, ident)

            kern_sb = const_pool.tile([E, M], dt.float32)
            nc.gpsimd.dma_start(out=kern_sb, in_=kern_d[:, :])
            cardT_sb = const_pool.tile([E, R], dt.float32)
            nc.gpsimd.dma_start(out=cardT_sb, in_=cardT_d[:, :])
            poolsT_sb = big_pool.tile([128, C], dt.float32)  # [d_in, d_out*B + b]
            nc.gpsimd.dma_start(out=poolsT_sb, in_=poolsT_d[:, :])

            x_sb = big_pool.tile([128, RC * M], dt.float32)  # [c_in, chunk*M + m]
            for k in range(RC):
                psx = psum_t.tile([128, M], dt.float32, tag="pst")
                nc.tensor.matmul(
                    psx,
                    cardT_sb[:, k * 128 : (k + 1) * 128],  # [e, c_in]
                    kern_sb[:, :],                          # [e, m]
                    start=True,
                    stop=True,
                )
                nc.vector.tensor_copy(out=x_sb[:, k * M : (k + 1) * M], in_=psx)

            # ---- Phases B+C: main accumulation + per-group finalization ----
            w_sb = big_pool.tile([128, ND * M], dt.float32)  # [d_in, d_out*M + m]
            out_ps = psum_o.tile([B, M], dt.float32)

            for g in range(NG):
                psa = psum_a.tile([128, 512], dt.float32)
                psb = psum_b.tile([128, 512], dt.float32)
                for ci in range(RC):
                    mp_t = mp_pool.tile([128, DG], dt.float32)
                    nc.sync.dma_start(
                        out=mp_t,
                        in_=mp_d[ci * 128 : (ci + 1) * 128, g * DG : (g + 1) * DG],
                    )
                    xt = x_sb[:, ci * M : (ci + 1) * M]
                    nc.tensor.matmul(
                        psa, xt, mp_t[:, 0:512],
                        start=(ci == 0), stop=(ci == RC - 1),
                    )
                    nc.tensor.matmul(
                        psb, xt, mp_t[:, 512:1024],
                        start=(ci == 0), stop=(ci == RC - 1),
                    )

                # evacuate W^T group to SBUF staging
                stage = stage_pool.tile([128, DG], dt.float32)  # [m, d_local]
                nc.vector.tensor_copy(out=stage[:, 0:512], in_=psa)
                nc.vector.tensor_copy(out=stage[:, 512:1024], in_=psb)

                # transpose to W layout and fold into the final contraction
                for t in range(TPG):
                    d_out = g * TPG + t
                    pst = psum_t.tile([128, 128], dt.float32, tag="pst")
                    nc.tensor.transpose(
                        pst, stage[:, t * 128 : (t + 1) * 128], ident
                    )
                    wt = w_sb[:, d_out * M : (d_out + 1) * M]
                    nc.vector.tensor_copy(out=wt, in_=pst)
                    nc.tensor.matmul(
                        out_ps,
                        poolsT_sb[:, d_out * 128 : (d_out + 1) * 128],  # [d_in, b]
                        wt,                                             # [d_in, m]
                        start=(d_out == 0),
                        stop=(d_out == ND - 1),
                        skip_group_check=True,
                    )

            out_sb = const_pool.tile([B, M], dt.float32)
            nc.vector.tensor_copy(out=out_sb, in_=out_ps)
            nc.sync.dma_start(out=out_d[:, :], in_=out_sb)

    nc.compile()
    _NC_CACHE["nc"] = nc
    return nc


def _prep_in_maps(batch_pools, metapath, card_embeddings, kern):
    batch_pools = np.ascontiguousarray(batch_pools, dtype=np.float32)
    metapath = np.ascontiguousarray(metapath, dtype=np.float32)
    card_embeddings = np.ascontiguousarray(card_embeddings, dtype=np.float32)
    kern = np.ascontiguousarray(kern, dtype=np.float32)

    # poolsT tiled: [d_in, d_out*B + b] = pools[b, d_out*128 + d_in]
    poolsT = np.ascontiguousarray(
        batch_pools.T.reshape(128, 128, B).transpose(1, 0, 2).reshape(128, C)
    )

    in_maps = []
    for i in range(N_CORES):
        sl = slice(i * R, (i + 1) * R)
        card_slice = card_embeddings[sl]
        cardT = np.ascontiguousarray(
            card_slice.reshape(RC, 128, E).transpose(2, 0, 1).reshape(E, R)
        )
        in_maps.append(
            {
                "mp": metapath[sl],
                "poolsT": poolsT,
                "cardT": cardT,
                "kern": kern,
            }
        )
    return in_maps


def _run(inputs, **spmd_kwargs):
    from concourse.bass_utils import run_bass_kernel_spmd

    nc = _build_nc()
    in_maps = _prep_in_maps(
        inputs["batch_pools"],
        inputs["metapath"],
        inputs["card_embeddings"],
        inputs["kernel"],
    )
    res = run_bass_kernel_spmd(nc, in_maps, core_ids=list(range(N_CORES)), **spmd_kwargs)
    acc = np.zeros((B, M), dtype=np.float64)
    for r in res.results:
        acc += r["out"].astype(np.float64)
    return acc.astype(np.float32), res


def kernel(**inputs):
    out, _ = _run(inputs)
    return out
